# revision 23
# baseline (speedup 1.0000x reference)
"""Trainium2 Bass kernel for nn_EvenOddConvLayer (gnn_message_passing).

Sharding: nodes split across 8 cores (graph parallel). Node tables
(even|odd concatenated) are replicated to every core in DRAM; each core
gathers its neighbors' rows with indirect DMA, so the "all-gather" is
implicit. All dense/elementwise math runs feature-major on-chip; the
only layout transforms are host-side transposes (packing) and one PE
transpose per gathered 128-pair chunk.

kernel(**inputs) takes the FULL unsharded inputs and returns the full
(even_out, odd_out) tuple like the reference.
"""

import numpy as np

import concourse.bacc as bacc
import concourse.mybir as mybir
from concourse.bass import IndirectOffsetOnAxis
from concourse.bass_utils import run_bass_kernel_spmd
from concourse.tile import TileContext

# Problem shape (hardcoded per harness contract)
N, M, EVEN, ODD, EDGE = 50000, 20, 64, 64, 32
NCORES = 8
NSHARD = N // NCORES          # 6250 nodes per core
NT = 32                       # nodes per tile
NTILES = 196
NPAD = NT * NTILES            # 6272 padded nodes per core
T = NT * M                    # 640 pairs per tile
NCH = T // 128                # 5 gather chunks per tile
NCHUNKS = NTILES * NCH        # 980
GROUP = 7                     # tiles per ACT table-set phase group
PEI_CH = 448                  # per-node precompute chunk (14 chunks)
F32 = mybir.dt.float32
I32 = mybir.dt.int32

AF = mybir.ActivationFunctionType
ALU = mybir.AluOpType

_PROG = None


def _build_program():
    nc = bacc.Bacc("TRN2", target_bir_lowering=False, debug=False)

    # ---- DRAM I/O ----
    nodes_cat = nc.dram_tensor("nodes_cat", [N, 2 * EVEN], F32, kind="ExternalInput")
    idx_cols = nc.dram_tensor("idx_cols", [128, NCHUNKS], I32, kind="ExternalInput")
    nbrT = nc.dram_tensor("nbrT", [EDGE, NPAD * M], F32, kind="ExternalInput")
    evenT_dram = nc.dram_tensor("evenT_ones", [65, NPAD], F32, kind="ExternalInput")
    oddT_dram = nc.dram_tensor("oddT", [64, NPAD], F32, kind="ExternalInput")
    s32_dram = nc.dram_tensor("s32", [128, T], F32, kind="ExternalInput")
    ident_dram = nc.dram_tensor("ident", [128, 128], F32, kind="ExternalInput")
    w_m1_d = nc.dram_tensor("w_m1", [64, 128], F32, kind="ExternalInput")
    w_m2_d = nc.dram_tensor("w_m2", [96, 128], F32, kind="ExternalInput")
    w_cat_i_d = nc.dram_tensor("w_cat_i", [65, 128], F32, kind="ExternalInput")
    w_pei_b_d = nc.dram_tensor("w_pei_b", [65, 64], F32, kind="ExternalInput")
    w_pej_d = nc.dram_tensor("w_pej", [64, 64], F32, kind="ExternalInput")
    w_eg_d = nc.dram_tensor("w_eg", [64, 64], F32, kind="ExternalInput")
    w_em2_d = nc.dram_tensor("w_em2", [64, 64], F32, kind="ExternalInput")
    # W_og lives at partitions 64-127: its matmul streams H's bottom half
    # into PE rows 64-127, and lhsT must sit on the same partitions
    w_og_d = nc.dram_tensor("w_og", [128, 64], F32, kind="ExternalInput")
    w_om1_d = nc.dram_tensor("w_om1", [128, 64], F32, kind="ExternalInput")
    b_pej_d = nc.dram_tensor("b_pej", [64, 1], F32, kind="ExternalInput")
    b_y_d = nc.dram_tensor("b_y", [128, 1], F32, kind="ExternalInput")
    b_em2_d = nc.dram_tensor("b_em2", [64, 1], F32, kind="ExternalInput")
    outT = nc.dram_tensor("outT", [128, NPAD], F32, kind="ExternalOutput")

    with TileContext(nc) as tc:
        # ---- resident SBUF ----
        with tc.tile_pool(name="resident", bufs=1) as rp:
            idx_sb = rp.tile_from(idx_cols[:, :], name="idx_sb")
            s32 = rp.tile_from(s32_dram[:, :], name="s32_sb")
            ident = rp.tile_from(ident_dram[:, :], name="ident_sb")
            w_m1 = rp.tile_from(w_m1_d[:, :], name="w_m1_sb")
            w_m2 = rp.tile_from(w_m2_d[:, :], name="w_m2_sb")
            w_pej = rp.tile_from(w_pej_d[:, :], name="w_pej_sb")
            w_eg = rp.tile_from(w_eg_d[:, :], name="w_eg_sb")
            w_em2 = rp.tile_from(w_em2_d[:, :], name="w_em2_sb")
            w_og = rp.tile_from(w_og_d[:, :], name="w_og_sb")
            w_om1 = rp.tile_from(w_om1_d[:, :], name="w_om1_sb")
            b_pej = rp.tile_from(b_pej_d[:, :], name="b_pej_sb")
            b_y = rp.tile_from(b_y_d[:, :], name="b_y_sb")
            b_em2 = rp.tile_from(b_em2_d[:, :], name="b_em2_sb")
            # C = [oddT (p0-63) ; pei (p64-127)], feature-major per own node
            C = rp.tile([128, NPAD], F32, tag="C_res")
            # AiGi node-major: node n -> partition n%128, free block n//128
            aigi = rp.tile([128, NPAD], F32, tag="aigi_res")
            outbuf = rp.tile([128, NPAD], F32, tag="outbuf_res")

            nc.sync.dma_start(C[0:64, :], oddT_dram[:, :])

            # ---- per-node precompute: pei (feature-major) and AiGi (node-major) ----
            with (
                tc.tile_pool(name="pre_sb", bufs=1) as pp,
                tc.tile_pool(name="pre_ps", bufs=2, space="PSUM") as ppp,
            ):
                evenT = pp.tile([65, NPAD], F32, tag="evenT")
                nc.sync.dma_start(evenT[:, :], evenT_dram[:, :])
                w_cat_i = pp.tile_from(w_cat_i_d[:, :], name="w_cat_i_sb", bufs=1)
                w_pei_b = pp.tile_from(w_pei_b_d[:, :], name="w_pei_b_sb", bufs=1)
                # pei[f, n] = (even[n] @ W_pei + b_pei)[f]  (bias via ones row)
                # written at partitions 64-127 so downstream bases line up
                for c in range(NPAD // PEI_CH):
                    ps = ppp.tile([128, PEI_CH], F32, tag="pei_ps")
                    sl = slice(c * PEI_CH, (c + 1) * PEI_CH)
                    nc.tensor.matmul(ps[64:128, :], w_pei_b[:, :], evenT[:, sl],
                                     start=True, stop=True,
                                     tile_position=(0, 64))
                    nc.vector.tensor_copy(C[64:128, sl], ps[64:128, :])
                # AiGi_nm[n, :] = even[n] @ [W_em1_i | W_ogh_i] + [b_em1 | b_ogh]
                for c in range(NPAD // 128):
                    ps = ppp.tile([128, 128], F32, tag="aigi_ps")
                    sl = slice(c * 128, (c + 1) * 128)
                    nc.tensor.matmul(ps[:, :], evenT[:, sl], w_cat_i[:, :],
                                     start=True, stop=True)
                    nc.scalar.activation(aigi[:, sl], ps[:, :], AF.Copy)

            # ---- main loop pools ----
            with (
                tc.tile_pool(name="gbuf_p", bufs=2) as gp,
                tc.tile_pool(name="gt_p", bufs=2) as gtp,
                tc.tile_pool(name="m2r_p", bufs=2) as m2p,
                tc.tile_pool(name="h_p", bufs=2) as hp,
                tc.tile_pool(name="om1r_p", bufs=2) as op_,
                tc.tile_pool(name="stage_p", bufs=GROUP + 1) as stp,
                tc.tile_pool(name="sgpr_p", bufs=2) as sgp,
                tc.tile_pool(name="ps_early", bufs=1, space="PSUM") as pse,
                tc.tile_pool(name="ps_p1", bufs=1, space="PSUM") as ps1,
                tc.tile_pool(name="ps_y", bufs=1, space="PSUM") as psy,
                tc.tile_pool(name="ps_z", bufs=1, space="PSUM") as psz,
            ):
                ngroups = (NTILES + GROUP - 1) // GROUP
                for g in range(ngroups):
                    tiles = range(g * GROUP, min((g + 1) * GROUP, NTILES))
                    staged = {}
                    # ---------- phase A (softplus table set) ----------
                    for t in tiles:
                        nsl = slice(t * NT, (t + 1) * NT)        # node cols
                        psl = slice(t * T, (t + 1) * T)          # pair cols
                        gbuf = gp.tile([128, T], F32, tag="gbuf")
                        # one SWDGE op per 128-row chunk: the HW DGE only
                        # supports [P, 1]-shaped offset APs
                        for k in range(NCH):
                            nc.gpsimd.indirect_dma_start(
                                out=gbuf[:, k * 128:(k + 1) * 128],
                                out_offset=None,
                                in_=nodes_cat[:, :],
                                in_offset=IndirectOffsetOnAxis(
                                    ap=idx_sb[:, t * NCH + k:t * NCH + k + 1],
                                    axis=0),
                            )
                        m2r = m2p.tile([96, T], F32, tag="m2r")
                        nc.sync.dma_start(m2r[64:96, :], nbrT[:, psl])

                        ttr = pse.tile([128, T], F32, tag="early")
                        for k in range(NCH):
                            ksl = slice(k * 128, (k + 1) * 128)
                            nc.tensor.transpose(ttr[:, ksl], gbuf[:, ksl],
                                                ident[:, :])
                        # only even_jT needs to reach SBUF (matmul rhs);
                        # odd_jT is consumed straight from the transpose PSUM
                        gt = gtp.tile([64, T], F32, tag="gt")
                        nc.vector.tensor_copy(gt[:, :], ttr[0:64, :])

                        # cross = odd_i * odd_j -> m2r rows 0-63
                        nc.vector.tensor_mul(
                            m2r[0:64, :].rearrange("p (n m) -> p n m", m=M),
                            C[0:64, nsl].to_broadcast([64, NT, M]),
                            ttr[64:128, :].rearrange("p (n m) -> p n m", m=M),
                        )
                        # ei = pei * odd_j; reads odd_jT from the transpose
                        # PSUM, so it must precede the pej tile (shared slot)
                        om1r = op_.tile([128, T], F32, tag="om1r")
                        nc.vector.tensor_mul(
                            om1r[64:128, :].rearrange("p (n m) -> p n m", m=M),
                            ttr[64:128, :].rearrange("p (n m) -> p n m", m=M),
                            C[64:128, nsl].to_broadcast([64, NT, M]))

                        # PSUM1 = AiGi(one-hot) + W_m1.T @ even_j + W_m2.T @ [cross;nbr]
                        p1 = ps1.tile([128, 1024], F32, tag="p1")
                        arow = 32 * (t % 4)
                        lhs_aigi = aigi[arow:arow + 32,
                                        (t // 4) * 128:(t // 4 + 1) * 128]
                        for h in range(2):
                            osl = slice(h * 512, h * 512 + 320)
                            hsl = slice(h * 320, (h + 1) * 320)
                            nc.tensor.matmul(p1[:, osl], lhs_aigi,
                                             s32[arow:arow + 32, hsl],
                                             start=True, stop=False,
                                             tile_position=(arow, 0))
                            nc.tensor.matmul(p1[:, osl], w_m1[:, :],
                                             gt[0:64, hsl], start=False, stop=False)
                            nc.tensor.matmul(p1[:, osl], w_m2[:, :],
                                             m2r[:, hsl], start=False, stop=True)
                        p1v = p1[:, :].rearrange("p (b x) -> p b x", x=512)[:, :, 0:320]
                        hh = hp.tile([128, T], F32, tag="hh")
                        # softplus(x) = ln(exp(x) + 1): both fns in the
                        # natural_log_exp table set (no native softplus)
                        e1 = hp.tile([128, T], F32, tag="e1")
                        nc.scalar.activation(
                            e1[:, :].rearrange("p (b x) -> p b x", x=320),
                            p1v, AF.Exp)
                        nc.scalar.activation(hh[:, :], e1[:, :], AF.Ln, bias=1.0)

                        # pej (no bias yet) -> psum, then ie/ei into om1 rhs
                        pej = pse.tile([64, 1024], F32, tag="early")
                        for h in range(2):
                            nc.tensor.matmul(pej[:, h * 512:h * 512 + 320],
                                             w_pej[:, :],
                                             gt[:, h * 320:(h + 1) * 320],
                                             start=True, stop=True)
                        pejv = pej[:, :].rearrange("p (b x) -> p b x", x=512)[:, :, 0:320]
                        # ie = (pej + b_pej) * odd_i
                        nc.vector.scalar_tensor_tensor(
                            om1r[0:64, :].rearrange("p (b x) -> p b x", x=320),
                            pejv, b_pej[:, :],
                            C[0:64, nsl].to_broadcast([64, NT, M]),
                            op0=ALU.add, op1=ALU.mult)

                        # late matmuls: Y = [eg; og], Z = [em2; om1]
                        yps = psy.tile([128, 1024], F32, tag="yps")
                        zps = psz.tile([128, 1024], F32, tag="zps")
                        for h in range(2):
                            osl = slice(h * 512, h * 512 + 320)
                            hsl = slice(h * 320, (h + 1) * 320)
                            nc.tensor.matmul(yps[0:64, osl], w_eg[:, :],
                                             hh[0:64, hsl], start=True, stop=True,
                                             tile_position=(0, 0))
                            nc.tensor.matmul(yps[64:128, osl], w_og[64:128, :],
                                             hh[64:128, hsl], start=True, stop=True,
                                             tile_position=(64, 64))
                            nc.tensor.matmul(zps[0:64, osl], w_em2[:, :],
                                             hh[0:64, hsl], start=True, stop=True,
                                             tile_position=(0, 0))
                            nc.tensor.matmul(zps[64:128, osl], w_om1[:, :],
                                             om1r[:, hsl], start=True, stop=True,
                                             tile_position=(0, 64))
                        ypsv = yps[:, :].rearrange("p (b x) -> p b x", x=512)[:, :, 0:320]
                        zpsv = zps[:, :].rearrange("p (b x) -> p b x", x=512)[:, :, 0:320]

                        sv = stp.tile([128, T], F32, tag="sv")
                        # em2 softplus = ln(exp(z + b) + 1)
                        e2 = hp.tile([64, T], F32, tag="e2")
                        nc.scalar.activation(
                            e2[:, :].rearrange("p (b x) -> p b x", x=320),
                            zpsv[0:64], AF.Exp, bias=b_em2[:, :])
                        nc.scalar.activation(sv[0:64, :], e2[:, :], AF.Ln,
                                             bias=1.0)
                        # stage Y and Z[om1] to SBUF for phase B
                        yraw = stp.tile([128, T], F32, tag="yraw")
                        nc.vector.tensor_copy(
                            yraw[:, :].rearrange("p (b x) -> p b x", x=320), ypsv)
                        zraw = stp.tile([128, T], F32, tag="zraw")
                        nc.scalar.activation(
                            zraw[64:128, :].rearrange("p (b x) -> p b x", x=320),
                            zpsv[64:128], AF.Copy)
                        staged[t] = (sv, yraw, zraw)

                    # ---------- phase B (sigmoid/tanh table set) ----------
                    for t in tiles:
                        sv, yraw, zraw = staged[t]
                        sg = sgp.tile([128, T], F32, tag="sg")
                        nc.scalar.activation(sg[:, :], yraw[:, :], AF.Sigmoid,
                                             bias=b_y[:, :])
                        nc.scalar.activation(sv[64:128, :], zraw[64:128, :],
                                             AF.Tanh)
                        pr = sgp.tile([128, T], F32, tag="pr")
                        nc.vector.tensor_mul(pr[:, :], sg[:, :], sv[:, :])
                        nc.vector.reduce_sum(
                            outbuf[:, t * NT:(t + 1) * NT],
                            pr[:, :].rearrange("p (n m) -> p n m", m=M),
                            axis=mybir.AxisListType.X)

                # ---- residual add + store ----
                with tc.tile_pool(name="res_p", bufs=2) as resp:
                    for c in range(NPAD // PEI_CH):
                        sl = slice(c * PEI_CH, (c + 1) * PEI_CH)
                        tmp = resp.tile([128, PEI_CH], F32, tag="res_tmp")
                        nc.sync.dma_start(tmp[0:64, :], evenT_dram[0:64, sl])
                        nc.sync.dma_start(tmp[64:128, :], oddT_dram[:, sl])
                        nc.vector.tensor_add(outbuf[0:64, sl], outbuf[0:64, sl],
                                             tmp[0:64, :])
                        nc.vector.tensor_add(outbuf[64:128, sl],
                                             outbuf[64:128, sl], tmp[64:128, :])
                nc.sync.dma_start(outT[:, :], outbuf[:, :])

    nc.compile()
    return nc


def _get_program():
    global _PROG
    if _PROG is None:
        _PROG = _build_program()
    return _PROG


def _host_prep(even, odd, nbr_fea, idx, W_em1, b_em1, W_eg, b_eg, W_em2, b_em2,
               W_pej, b_pej, W_pei, b_pei, W_om1, W_ogh, b_ogh, W_og, b_og):
    f32 = np.float32
    nodes_cat = np.ascontiguousarray(
        np.concatenate([even, odd], axis=1), dtype=f32)          # [N, 128]
    # weights, stacked for the fused heads (cols: [em1 | ogh])
    w_m1 = np.ascontiguousarray(
        np.concatenate([W_em1[64:128], W_ogh[64:128]], 1), f32)  # even_j rows
    w_m2 = np.ascontiguousarray(np.concatenate([
        np.concatenate([W_em1[160:224], W_ogh[160:224]], 1),     # cross rows 0-63
        np.concatenate([W_em1[128:160], W_ogh[128:160]], 1),     # nbr rows 64-95
    ], 0), f32)
    w_cat_i = np.ascontiguousarray(np.concatenate([
        np.concatenate([W_em1[0:64], W_ogh[0:64]], 1),
        np.concatenate([b_em1[None, :], b_ogh[None, :]], 1),
    ], 0), f32)                                                  # [65, 128]
    w_pei_b = np.ascontiguousarray(
        np.concatenate([W_pei, b_pei[None, :]], 0), f32)         # [65, 64]
    s32 = (np.arange(T)[None, :] // M == np.arange(128)[:, None] % NT
           ).astype(f32)
    ident = np.eye(128, dtype=f32)
    b_y = np.concatenate([b_eg, b_og])[:, None].astype(f32)

    common = dict(
        nodes_cat=nodes_cat, s32=s32, ident=ident,
        w_m1=w_m1, w_m2=w_m2, w_cat_i=w_cat_i, w_pei_b=w_pei_b,
        w_pej=np.ascontiguousarray(W_pej, f32),
        w_eg=np.ascontiguousarray(W_eg, f32),
        w_em2=np.ascontiguousarray(W_em2, f32),
        w_og=np.ascontiguousarray(
            np.concatenate([np.zeros((64, 64), f32), W_og], 0)),
        w_om1=np.ascontiguousarray(W_om1, f32),
        b_pej=np.ascontiguousarray(b_pej[:, None], f32),
        b_y=b_y,
        b_em2=np.ascontiguousarray(b_em2[:, None], f32),
    )

    in_maps = []
    for c in range(NCORES):
        sl = slice(c * NSHARD, (c + 1) * NSHARD)
        ev = np.zeros((NPAD, EVEN), f32); ev[:NSHARD] = even[sl]
        od = np.zeros((NPAD, ODD), f32); od[:NSHARD] = odd[sl]
        nb = np.zeros((NPAD, M, EDGE), f32); nb[:NSHARD] = nbr_fea[sl]
        ix = np.zeros((NPAD, M), np.int32); ix[:NSHARD] = idx[sl]
        evenT_ones = np.ones((65, NPAD), f32)
        evenT_ones[0:64] = ev.T
        in_maps.append(dict(
            common,
            evenT_ones=np.ascontiguousarray(evenT_ones),
            oddT=np.ascontiguousarray(od.T),
            nbrT=np.ascontiguousarray(nb.reshape(NPAD * M, EDGE).T),
            idx_cols=np.ascontiguousarray(
                ix.reshape(NCHUNKS, 128).T),
        ))
    return in_maps


def kernel(even_node, odd_node, nbr_fea, nbr_fea_idx,
           W_em1, b_em1, W_eg, b_eg, W_em2, b_em2,
           W_pej, b_pej, W_pei, b_pei, W_om1,
           W_ogh, b_ogh, W_og, b_og):
    even = np.asarray(even_node, np.float32)
    odd = np.asarray(odd_node, np.float32)
    nbr = np.asarray(nbr_fea, np.float32)
    idx = np.asarray(nbr_fea_idx).astype(np.int32)
    args = [np.asarray(a, np.float32) for a in (
        W_em1, b_em1, W_eg, b_eg, W_em2, b_em2, W_pej, b_pej,
        W_pei, b_pei, W_om1, W_ogh, b_ogh, W_og, b_og)]
    (W_em1, b_em1, W_eg, b_eg, W_em2, b_em2, W_pej, b_pej,
     W_pei, b_pei, W_om1, W_ogh, b_ogh, W_og, b_og) = args

    nc = _get_program()
    in_maps = _host_prep(even, odd, nbr, idx, W_em1, b_em1, W_eg, b_eg,
                         W_em2, b_em2, W_pej, b_pej, W_pei, b_pei, W_om1,
                         W_ogh, b_ogh, W_og, b_og)
    res = run_bass_kernel_spmd(nc, in_maps, list(range(NCORES)))
    even_out = np.concatenate(
        [r["outT"][0:64, :NSHARD].T for r in res.results], 0)
    odd_out = np.concatenate(
        [r["outT"][64:128, :NSHARD].T for r in res.results], 0)
    return even_out.astype(np.float32), odd_out.astype(np.float32)
